# revision 5
# baseline (speedup 1.0000x reference)
"""Trainium2 Bass kernel for CustomConv1d.

Problem: y = conv1d(x, weight, bias), x [32, 256, 4096] f32,
weight [256, 256, 5] f32, bias [256] f32, stride 1, pad 2.

Strategy: data-parallel over batch across 8 NeuronCores (4 batches/core,
weights+bias broadcast, no collectives). Per core the conv is computed as
matmuls on the tensor engine: for each output-channel chunk (128) and each
512-wide output tile, accumulate 10 matmuls in PSUM (5 taps x 2 input-channel
chunks of 128):

  out[co, w] = sum_{k, ci} weight[co, ci, k] * xpad[ci, w + k]

All matmul operands are bf16 (host-converted): the fp32r path issues a
188ns LDWEIGHTS per matmul that exceeds the 213ns moving stream and caps
issue rate at ~233ns/matmul; bf16 LDWEIGHTS (~100ns) hides fully under the
stream so matmuls issue back-to-back at ~216ns. bf16 also halves x/w DMA
bytes. PSUM accumulation stays fp32; l2 rel err ~2.3e-3 (gate 2e-2).

DMA model (measured): a load costs ~21ns per row-packet on its ring, one
packet per destination partition (so any tile fill >= 128 x 21ns ~ 2.7us
of ring time, more if byte-bound); the sync and scalar HW rings dispatch
in parallel; stores (contiguous DRAM dst) are cheap. x is therefore
host-sliced into per-psum-tile 516-col halo slices [b, p, n, cic, 516]
(each moving operand is tile[:, n, cic, k:k+512]); batch 0 loads its
first two slices as individual 128-packet fills (the startup critical
path is just w_coc0 on sync parallel with slice-0 on scalar, ~11.5us),
then pairs, while batches 1-3 are ONE 128-packet DMA each. Ring order is
arranged so every tile lands just before its first matmul. Output is
written store-contiguous [b, coc, n, co, 512] (host inverse-transposes
the gathered result — host time is free). gpsimd's fragile SW DGE queue
gets only memset + bias. Warm-up matmuls bridge the PE clock ramp (the
clock survives idles up to ~2.5us, so the bridge need not be exact).
"""

import os

import numpy as np

try:
    import ml_dtypes

    BF16_NP = np.dtype(ml_dtypes.bfloat16)
except ImportError:  # pragma: no cover
    BF16_NP = None

import concourse.mybir as mybir
import concourse.tile as tile
from concourse import bacc
from concourse.bass_utils import run_bass_kernel_spmd


BF16 = mybir.dt.bfloat16
F32 = mybir.dt.float32

B, CIN, COUT, W, K, PAD = 32, 256, 256, 4096, 5, 2
NCORES = 8
BPC = B // NCORES          # batches per core
P = 128                    # partition dim
NT = 512                   # moving-operand tile (one fp32 PSUM bank)
N_CIC = CIN // P           # input-channel chunks
N_COC = COUT // P          # output-channel chunks
N_WT = W // NT             # output width tiles
HW_ = NT + 2 * PAD         # halo slice width per psum tile (516)
WELE = K * N_CIC * P       # weight elems per partition per coc (1280)
N_WARM = 5                 # PE clock-ramp matmuls while first DMAs land
NT_LAST_B = 128            # final-tile tail split: last psum group width


def _build_program():
    # Bacc (not plain Bass): its finalize() runs generate_event_semaphores,
    # which splits multi-sem waits into event-semaphore chains — the TRN2
    # walrus here accepts at most one sync wait per regular instruction.
    nc = bacc.Bacc()
    # x host-padded halo slices: xh[b, p, n, cic, j] = xpad[b, cic*128+p, n*512+j]
    x_d = nc.declare_dram_parameter("xh", [BPC, P, N_WT, N_CIC, HW_], BF16,
                                    isOutput=False)
    # weights host-transposed: wt[coc, ci, (k, cic, co)]
    wt_d = nc.declare_dram_parameter("wt", [N_COC, P, WELE], BF16, isOutput=False)
    b_d = nc.declare_dram_parameter("bias2", [P, N_COC], F32, isOutput=False)
    # output store-contiguous: o5[b, coc, n, co, j] = out[b, coc*P+co, n*NT+j]
    o_d = nc.declare_dram_parameter("out", [BPC, N_COC, N_WT, P, NT], F32, isOutput=True)

    with tile.TileContext(nc) as tc:
        with (
            tc.tile_pool(name="wpool", bufs=1) as wpool,
            tc.tile_pool(name="xpool", bufs=1) as xpool,
            tc.tile_pool(name="opool", bufs=2 * N_COC) as opool,
            tc.tile_pool(name="psum", bufs=8, space="PSUM") as pspool,
        ):
            # PE warm-up scratch (Tile insists it be written): memset on
            # vector, which is idle until the first bias-add ~5us later, so
            # the dummy matmuls below can start engaging the HAM clock-gate
            # right after the preamble barrier.
            warm = wpool.tile([P, NT], BF16)
            nc.vector.memset(warm[:], 0.0)

            def xtile(b, n0, n1, eng):
                t = xpool.tile([P, n1 - n0, N_CIC, HW_], BF16,
                               name=f"x{b}_{n0}")
                eng.dma_start(t[:], x_d[b, :, n0:n1])
                return t

            # Startup critical path: the first real matmul needs only the
            # k=0 weight chunks and slice-0/cic-0 of x. Packets only start
            # flowing ~1.5us (sync) / ~2.4us (scalar) after dma_start
            # issues, and a fill's duration is ~bytes/215GB/s, so the
            # smallest possible pieces go FIRST on sync (the
            # faster-spinning ring); everything else is ordered per ring in
            # consumption order with its deadline in mind.
            w_sb0 = wpool.tile([P, WELE], BF16, name="w0")
            nc.sync.dma_start(w_sb0[:, :2 * P], wt_d[0, :, :2 * P])
            t0a = xpool.tile([P, 1, 1, HW_], BF16, name="x0a")
            nc.sync.dma_start(t0a[:], x_d[0, :, 0:1, 0:1])   # slice0 cic0
            t0b = xpool.tile([P, 1, 1, HW_], BF16, name="x0b")
            nc.sync.dma_start(t0b[:], x_d[0, :, 0:1, 1:2])   # slice0 cic1
            nc.scalar.dma_start(w_sb0[:, 2 * P:], wt_d[0, :, 2 * P:])
            t1 = xpool.tile([P, 1, N_CIC, HW_], BF16, name="x1")
            nc.scalar.dma_start(t1[:], x_d[0, :, 1:2])       # slice 1
            t23 = xtile(0, 2, 4, nc.sync)             # sync:   x0 slices 2-3
            t45 = xtile(0, 4, 6, nc.scalar)           # scalar: x0 slices 4-5
            w_sb1 = wpool.tile([P, WELE], BF16, name="w1")
            nc.scalar.dma_start(w_sb1[:], wt_d[1])    # scalar: w1
            t67 = xtile(0, 6, 8, nc.sync)             # sync:   x0 slices 6-7
            x0parts = [None, t1, t23, t23, t45, t45, t67, t67]
            x0base = [0, 1, 2, 2, 4, 4, 6, 6]

            # bias2 host-transposed to [P, N_COC] -> single [128, 2] DMA
            b_sb = wpool.tile([P, N_COC], F32)
            nc.gpsimd.dma_start(b_sb[:], b_d[:])

            ps_warm = pspool.tile([P, NT], F32, tag="ps", name="ps_warm")
            for _ in range(N_WARM):
                nc.tensor.matmul(ps_warm[:], warm[:, 0:P], warm[:])

            def rhs_ap(xts, b, cic, n, k, lo=0, width=NT):
                """moving operand: xpad[b, cic*P:+P, n*NT+k+lo : +width]"""
                if b == 0:
                    if n == 0:
                        t = t0a if cic == 0 else t0b
                        return t[:, 0, 0, k + lo:k + lo + width]
                    return x0parts[n][:, n - x0base[n], cic,
                                      k + lo:k + lo + width]
                return xts[:, n, cic, k + lo:k + lo + width]

            xts = None
            for b in range(BPC):
                if b + 1 < BPC:
                    # one 128-packet DMA per batch, all on scalar: its ring
                    # is idle after the prologue, and keeping the 16.5KB
                    # fill packets off the store ring (sync) removes
                    # periodic store-completion jitter that back-pressures
                    # PSUM into ~58ns PE stalls
                    nxt = xpool.tile([P, N_WT, N_CIC, HW_], BF16, tag="x",
                                     bufs=2, name=f"x{b + 1}")
                    nc.scalar.dma_start(nxt[:], x_d[b + 1])
                else:
                    nxt = None

                last_pass = b == BPC - 1
                for coc in range(N_COC):
                    w_lhs = w_sb0 if coc == 0 else w_sb1
                    ot = opool.tile([P, W], F32, tag="o")
                    for n in range(N_WT):
                        very_last = last_pass and coc == N_COC - 1 and n == N_WT - 1
                        if very_last:
                            # Tail: split the final tile into a 384-col and
                            # a 128-col accumulation group in SEPARATE psum
                            # banks (PE-write + DVE-read of one bank is
                            # fatal), so only a [P,128] add + one small
                            # store DMA sit after the last matmul. Stores
                            # go on different queues; each add into its own
                            # tile (same-tile writes chain through a ~310ns
                            # semaphore).
                            splits = ((0, NT - NT_LAST_B, nc.scalar),
                                      (NT - NT_LAST_B, NT, nc.sync))
                            for lo, hi, eng in splits:
                                ps = pspool.tile([P, NT], F32, tag="ps",
                                                 name=f"ps_last{lo}")
                                idx = 0
                                for k in range(K):
                                    for cic in range(N_CIC):
                                        nc.tensor.matmul(
                                            ps[:, :hi - lo],
                                            w_lhs[:, (k * N_CIC + cic) * P:
                                                  (k * N_CIC + cic + 1) * P],
                                            rhs_ap(xts, b, cic, n, k, lo,
                                                   hi - lo),
                                            start=(idx == 0),
                                            stop=(idx == K * N_CIC - 1),
                                        )
                                        idx += 1
                                oh = opool.tile([P, hi - lo], F32, tag="olast",
                                                bufs=2, name=f"olast{lo}")
                                nc.vector.tensor_scalar_add(
                                    oh[:], ps[:, :hi - lo], b_sb[:, coc:coc + 1]
                                )
                                eng.dma_start(o_d[b, coc, n, :, lo:hi], oh[:])
                            continue
                        ps = pspool.tile([P, NT], F32, tag="ps", name=f"ps{b}_{coc}_{n}")
                        idx = 0
                        for k in range(K):
                            for cic in range(N_CIC):
                                nc.tensor.matmul(
                                    ps[:],
                                    w_lhs[:, (k * N_CIC + cic) * P:
                                          (k * N_CIC + cic + 1) * P],
                                    rhs_ap(xts, b, cic, n, k),
                                    start=(idx == 0),
                                    stop=(idx == K * N_CIC - 1),
                                )
                                idx += 1
                        nc.vector.tensor_scalar_add(
                            ot[:, n * NT:(n + 1) * NT], ps[:], b_sb[:, coc:coc + 1]
                        )
                        nc.sync.dma_start(
                            o_d[b, coc, n], ot[:, n * NT:(n + 1) * NT]
                        )
                xts = nxt
    nc.finalize()
    return nc


_NC_CACHE = []


def kernel(x, weight, bias):
    assert x.shape == (B, CIN, W) and weight.shape == (COUT, CIN, K)
    if not _NC_CACHE:
        _NC_CACHE.append(_build_program())
    nc = _NC_CACHE[0]

    # wt[coc, ci, (k, cic, co)] = weight[coc*128+co, cic*128+ci, k]
    wt = np.ascontiguousarray(
        weight.astype(np.float32)
        .transpose(1, 2, 0)                      # [ci_full, k, co_full]
        .reshape(N_CIC, P, K, N_COC, P)          # [cic, ci, k, coc, co]
        .transpose(3, 1, 2, 0, 4)                # [coc, ci, k, cic, co]
        .astype(BF16_NP)
        .reshape(N_COC, P, WELE)
    )
    bias2 = np.ascontiguousarray(bias.astype(np.float32).reshape(N_COC, P).T)
    xpad = np.pad(x.astype(np.float32), ((0, 0), (0, 0), (PAD, PAD))).astype(BF16_NP)
    # xh[b, p, n, cic, j] = xpad[b, cic*128 + p, n*512 + j]
    xh = np.empty((B, P, N_WT, N_CIC, HW_), dtype=BF16_NP)
    for n in range(N_WT):
        sl = xpad[:, :, n * NT:n * NT + HW_]               # [B, 256, 516]
        xh[:, :, n] = sl.reshape(B, N_CIC, P, HW_).transpose(0, 2, 1, 3)
    in_maps = [
        {
            "xh": np.ascontiguousarray(xh[i * BPC:(i + 1) * BPC]),
            "wt": wt,
            "bias2": bias2,
        }
        for i in range(NCORES)
    ]
    res = run_bass_kernel_spmd(
        nc,
        in_maps,
        list(range(NCORES)),
        trace=bool(int(os.environ.get("KERNEL_TRACE", "0"))),
    )
    kernel.last_results = res
    # o5[b, coc, n, co, j] -> out[b, coc*128+co, n*512+j]
    full = np.concatenate(
        [res.results[i]["out"] for i in range(NCORES)], axis=0
    )
    return np.ascontiguousarray(
        full.transpose(0, 1, 3, 2, 4).reshape(B, COUT, W)
    )



# revision 9
# speedup vs baseline: 1.0210x; 1.0210x over previous
"""Trainium2 Bass kernel for CustomConv1d.

Problem: y = conv1d(x, weight, bias), x [32, 256, 4096] f32,
weight [256, 256, 5] f32, bias [256] f32, stride 1, pad 2.

Strategy: data-parallel over batch across 8 NeuronCores (4 batches/core,
weights+bias broadcast, no collectives). Per core the conv is computed as
matmuls on the tensor engine: for each output-channel chunk (128) and each
512-wide output tile, accumulate 10 matmuls in PSUM (5 taps x 2 input-channel
chunks of 128):

  out[co, w] = sum_{k, ci} weight[co, ci, k] * xpad[ci, w + k]

All matmul operands are bf16 (host-converted): the fp32r path issues a
188ns LDWEIGHTS per matmul that exceeds the 213ns moving stream and caps
issue rate at ~233ns/matmul; bf16 LDWEIGHTS (~100ns) hides fully under the
stream so matmuls issue back-to-back at ~216ns. bf16 also halves x/w DMA
bytes. PSUM accumulation stays fp32; l2 rel err ~2.3e-3 (gate 2e-2).

DMA model (measured): a load costs ~21ns per row-packet on its ring, one
packet per destination partition (so any tile fill >= 128 x 21ns ~ 2.7us
of ring time, more if byte-bound); the sync and scalar HW rings dispatch
in parallel; stores (contiguous DRAM dst) are cheap. x is therefore
host-sliced into per-psum-tile 516-col halo slices [b, p, n, cic, 516]
(each moving operand is tile[:, n, cic, k:k+512]); batch 0 loads its
first two slices as individual 128-packet fills (the startup critical
path is just w_coc0 on sync parallel with slice-0 on scalar, ~11.5us),
then pairs, while batches 1-3 are ONE 128-packet DMA each. Ring order is
arranged so every tile lands just before its first matmul. Output is
written store-contiguous [b, coc, n, co, 512] (host inverse-transposes
the gathered result — host time is free). gpsimd's fragile SW DGE queue
gets only memset + bias. Warm-up matmuls bridge the PE clock ramp (the
clock survives idles up to ~2.5us, so the bridge need not be exact).
"""

import os

import numpy as np

try:
    import ml_dtypes

    BF16_NP = np.dtype(ml_dtypes.bfloat16)
except ImportError:  # pragma: no cover
    BF16_NP = None

import concourse.mybir as mybir
import concourse.tile as tile
from concourse import bacc
from concourse.bass_utils import run_bass_kernel_spmd


BF16 = mybir.dt.bfloat16
F32 = mybir.dt.float32

B, CIN, COUT, W, K, PAD = 32, 256, 256, 4096, 5, 2
NCORES = 8
BPC = B // NCORES          # batches per core
P = 128                    # partition dim
NT = 512                   # moving-operand tile (one fp32 PSUM bank)
N_CIC = CIN // P           # input-channel chunks
N_COC = COUT // P          # output-channel chunks
N_WT = W // NT             # output width tiles
HW_ = NT + 2 * PAD         # halo slice width per psum tile (516)
WELE = K * N_CIC * P       # weight elems per partition per coc (1280)
N_WARM = 7                 # PE clock-ramp matmuls while first DMAs land
NT_LAST_B = 128            # final-tile tail split: last psum group width


def _build_program():
    # Bacc (not plain Bass): its finalize() runs generate_event_semaphores,
    # which splits multi-sem waits into event-semaphore chains — the TRN2
    # walrus here accepts at most one sync wait per regular instruction.
    nc = bacc.Bacc()
    # x host-padded halo slices: xh[b, p, n, cic, j] = xpad[b, cic*128+p, n*512+j]
    x_d = nc.declare_dram_parameter("xh", [BPC, P, N_WT, N_CIC, HW_], BF16,
                                    isOutput=False)
    # weights host-transposed: wt[coc, ci, (k, cic, co)]
    wt_d = nc.declare_dram_parameter("wt", [N_COC, P, WELE], BF16, isOutput=False)
    b_d = nc.declare_dram_parameter("bias2", [P, N_COC], F32, isOutput=False)
    # output store-contiguous: o5[b, coc, n, co, j] = out[b, coc*P+co, n*NT+j]
    o_d = nc.declare_dram_parameter("out", [BPC, N_COC, N_WT, P, NT], F32, isOutput=True)

    with tile.TileContext(nc) as tc:
        with (
            tc.tile_pool(name="wpool", bufs=1) as wpool,
            tc.tile_pool(name="xpool", bufs=1) as xpool,
            tc.tile_pool(name="opool", bufs=2 * N_COC) as opool,
            tc.tile_pool(name="psum", bufs=8, space="PSUM") as pspool,
        ):
            # PE warm-up scratch (Tile insists it be written): memset on
            # gpsimd, whose queue is free early. The dummy matmuls below keep
            # the HAM clock-gate busy while the prologue DMAs land.
            warm = wpool.tile([P, NT], BF16)
            nc.gpsimd.memset(warm[:], 0.0)

            def xtile(b, n0, n1, eng):
                t = xpool.tile([P, n1 - n0, N_CIC, HW_], BF16,
                               name=f"x{b}_{n0}")
                eng.dma_start(t[:], x_d[b, :, n0:n1])
                return t

            # Startup critical path. Measured DMA cost model: ~0.7us of
            # engine time per dma_start instruction, ~0.7us queue handoff
            # per descriptor, then bytes/215GB/s of transfer — so FEW BIG
            # fills win, and each ring's first descriptor only starts
            # moving ~1.5us (sync) / ~2.4us (scalar) after issue. The
            # first real matmul needs w0 and x slice-pair 0-1: the larger
            # piece (t01, 528KB) goes first on the faster-spinning sync
            # ring, w0 (327KB) first on scalar; both land ~10.5us.
            t01 = xtile(0, 0, 2, nc.sync)             # sync:   x0 slices 0-1
            w_sb0 = wpool.tile([P, WELE], BF16, name="w0")
            nc.scalar.dma_start(w_sb0[:], wt_d[0])    # scalar: w0
            t23 = xtile(0, 2, 4, nc.sync)             # sync:   x0 slices 2-3
            t45 = xtile(0, 4, 6, nc.scalar)           # scalar: x0 slices 4-5
            w_sb1 = wpool.tile([P, WELE], BF16, name="w1")
            nc.scalar.dma_start(w_sb1[:], wt_d[1])    # scalar: w1
            t67 = xtile(0, 6, 8, nc.sync)             # sync:   x0 slices 6-7
            x0parts = [t01, t01, t23, t23, t45, t45, t67, t67]
            x0base = [0, 0, 2, 2, 4, 4, 6, 6]

            # bias2 host-transposed to [P, N_COC] -> single [128, 2] DMA
            b_sb = wpool.tile([P, N_COC], F32)
            nc.gpsimd.dma_start(b_sb[:], b_d[:])

            ps_warm = pspool.tile([P, NT], F32, tag="ps", name="ps_warm")
            for _ in range(N_WARM):
                nc.tensor.matmul(ps_warm[:], warm[:, 0:P], warm[:])

            def rhs_ap(xts, b, cic, n, k, lo=0, width=NT):
                """moving operand: xpad[b, cic*P:+P, n*NT+k+lo : +width]"""
                if b == 0:
                    return x0parts[n][:, n - x0base[n], cic,
                                      k + lo:k + lo + width]
                return xts[:, n, cic, k + lo:k + lo + width]

            xts = None
            for b in range(BPC):
                if b + 1 < BPC:
                    # one 128-packet DMA per batch; b1 behind the prologue
                    # on scalar, b2 on sync, b3 on scalar
                    nxt = xpool.tile([P, N_WT, N_CIC, HW_], BF16, tag="x",
                                     bufs=2, name=f"x{b + 1}")
                    eng = nc.sync if b % 2 else nc.scalar
                    eng.dma_start(nxt[:], x_d[b + 1])
                else:
                    nxt = None

                last_pass = b == BPC - 1
                for coc in range(N_COC):
                    w_lhs = w_sb0 if coc == 0 else w_sb1
                    ot = opool.tile([P, W], F32, tag="o")
                    for n in range(N_WT):
                        very_last = last_pass and coc == N_COC - 1 and n == N_WT - 1
                        if very_last:
                            # Tail: split the final tile into a 384-col and
                            # a 128-col accumulation group in SEPARATE psum
                            # banks (PE-write + DVE-read of one bank is
                            # fatal), so only a [P,128] add + one small
                            # store DMA sit after the last matmul. Stores
                            # go on different queues; each add into its own
                            # tile (same-tile writes chain through a ~310ns
                            # semaphore).
                            splits = ((0, NT - NT_LAST_B, nc.scalar),
                                      (NT - NT_LAST_B, NT, nc.sync))
                            for lo, hi, eng in splits:
                                ps = pspool.tile([P, NT], F32, tag="ps",
                                                 name=f"ps_last{lo}")
                                idx = 0
                                for k in range(K):
                                    for cic in range(N_CIC):
                                        nc.tensor.matmul(
                                            ps[:, :hi - lo],
                                            w_lhs[:, (k * N_CIC + cic) * P:
                                                  (k * N_CIC + cic + 1) * P],
                                            rhs_ap(xts, b, cic, n, k, lo,
                                                   hi - lo),
                                            start=(idx == 0),
                                            stop=(idx == K * N_CIC - 1),
                                        )
                                        idx += 1
                                oh = opool.tile([P, hi - lo], F32, tag="olast",
                                                bufs=2, name=f"olast{lo}")
                                nc.vector.tensor_scalar_add(
                                    oh[:], ps[:, :hi - lo], b_sb[:, coc:coc + 1]
                                )
                                eng.dma_start(o_d[b, coc, n, :, lo:hi], oh[:])
                            continue
                        ps = pspool.tile([P, NT], F32, tag="ps", name=f"ps{b}_{coc}_{n}")
                        idx = 0
                        for k in range(K):
                            for cic in range(N_CIC):
                                nc.tensor.matmul(
                                    ps[:],
                                    w_lhs[:, (k * N_CIC + cic) * P:
                                          (k * N_CIC + cic + 1) * P],
                                    rhs_ap(xts, b, cic, n, k),
                                    start=(idx == 0),
                                    stop=(idx == K * N_CIC - 1),
                                )
                                idx += 1
                        nc.vector.tensor_scalar_add(
                            ot[:, n * NT:(n + 1) * NT], ps[:], b_sb[:, coc:coc + 1]
                        )
                        nc.sync.dma_start(
                            o_d[b, coc, n], ot[:, n * NT:(n + 1) * NT]
                        )
                xts = nxt
    nc.finalize()
    return nc


_NC_CACHE = []


def kernel(x, weight, bias):
    assert x.shape == (B, CIN, W) and weight.shape == (COUT, CIN, K)
    if not _NC_CACHE:
        _NC_CACHE.append(_build_program())
    nc = _NC_CACHE[0]

    # wt[coc, ci, (k, cic, co)] = weight[coc*128+co, cic*128+ci, k]
    wt = np.ascontiguousarray(
        weight.astype(np.float32)
        .transpose(1, 2, 0)                      # [ci_full, k, co_full]
        .reshape(N_CIC, P, K, N_COC, P)          # [cic, ci, k, coc, co]
        .transpose(3, 1, 2, 0, 4)                # [coc, ci, k, cic, co]
        .astype(BF16_NP)
        .reshape(N_COC, P, WELE)
    )
    bias2 = np.ascontiguousarray(bias.astype(np.float32).reshape(N_COC, P).T)
    xpad = np.pad(x.astype(np.float32), ((0, 0), (0, 0), (PAD, PAD))).astype(BF16_NP)
    # xh[b, p, n, cic, j] = xpad[b, cic*128 + p, n*512 + j]
    xh = np.empty((B, P, N_WT, N_CIC, HW_), dtype=BF16_NP)
    for n in range(N_WT):
        sl = xpad[:, :, n * NT:n * NT + HW_]               # [B, 256, 516]
        xh[:, :, n] = sl.reshape(B, N_CIC, P, HW_).transpose(0, 2, 1, 3)
    in_maps = [
        {
            "xh": np.ascontiguousarray(xh[i * BPC:(i + 1) * BPC]),
            "wt": wt,
            "bias2": bias2,
        }
        for i in range(NCORES)
    ]
    res = run_bass_kernel_spmd(
        nc,
        in_maps,
        list(range(NCORES)),
        trace=bool(int(os.environ.get("KERNEL_TRACE", "0"))),
    )
    kernel.last_results = res
    # o5[b, coc, n, co, j] -> out[b, coc*128+co, n*512+j]
    full = np.concatenate(
        [res.results[i]["out"] for i in range(NCORES)], axis=0
    )
    return np.ascontiguousarray(
        full.transpose(0, 1, 3, 2, 4).reshape(B, COUT, W)
    )

